# revision 1
# baseline (speedup 1.0000x reference)
"""Causal self-attention (B=2, T=2048, C=2048, H=16, rope) on 8 trn2 cores.

Sharding: tensor-parallel over heads. Each core owns 2 of 16 heads:
  - Wqkv columns for its heads (q,k,v), Wproj rows for its heads.
  - Computes qkv projection, rope, causal attention, and its partial
    output projection y_c = O_c @ Wproj_c  (full [4096, 2048]).
  - Host sums the 8 partials (the all-reduce / unshard for row-parallel TP).

All matmul operands fp16 (PE full rate; fp32 is 1/4 rate), fp32 PSUM
accumulation. Layouts keep the pipeline transpose-free except one 128x128
PE transpose per output tile (O -> O^T for the projection). Softmax sums
come free as a ones-column appended to V; normalization is applied to O
before projection. exp runs on the scalar engine over paired 2-bank PSUM
tiles; diagonal tiles are restricted to their valid causal column range.
"""

import sys

for _p in ("/opt/trn_rl_repo",):
    if _p not in sys.path:
        sys.path.append(_p)

import numpy as np

# ---- problem constants (hardcoded per the task contract) ----
B, T, C, H = 2, 2048, 2048, 16
D = C // H  # 128
NCORES = 8
HPC = H // NCORES  # heads per core = 2
NTOK = B * T  # 4096
P = 128
CT = C // P  # 16 contraction tiles
TOK512 = NTOK // 512  # 8
NQ = T // 512  # q-tiles per unit = 4
TT = NTOK // P  # 32 token 128-tiles
SCALE = 1.0 / np.sqrt(D)

_compiled = None

# tuning knobs (overridable before _build_bass for experiments)
KNOBS = {"cs": 2, "co": 4, "work": 2, "pt": 6, "ysb": 8, "rope": 4, "osb": 8, "xt": 2, "dact": 2, "oact": 1, "otact": 1}


def _build_bass():
    import concourse.bacc as bacc
    import concourse.mybir as mybir
    import concourse.tile as tile
    from contextlib import ExitStack

    f16 = mybir.dt.float16
    f32 = mybir.dt.float32
    Exp = mybir.ActivationFunctionType.Exp

    nc = bacc.Bacc()

    xT = nc.declare_dram_parameter("xT", [C, NTOK], f16, isOutput=False)
    wqk = nc.declare_dram_parameter("wqk", [C, 2 * HPC * D], f16, isOutput=False)
    wv = nc.declare_dram_parameter("wv", [C, HPC * D], f16, isOutput=False)
    wproj = nc.declare_dram_parameter("wproj", [HPC * D, C], f16, isOutput=False)
    cos_t = nc.declare_dram_parameter("cos_t", [P, NTOK], f16, isOutput=False)
    sin_t = nc.declare_dram_parameter("sin_t", [P, NTOK], f16, isOutput=False)
    maskw = nc.declare_dram_parameter("maskw", [P, 1024], f16, isOutput=False)
    ident = nc.declare_dram_parameter("ident", [P, P], f16, isOutput=False)
    rotmp = nc.declare_dram_parameter("rotm", [P, P], f16, isOutput=False)
    y = nc.declare_dram_parameter("y", [NTOK, C], f16, isOutput=True)

    with tile.TileContext(nc) as tc, ExitStack() as ctx:
        pers = ctx.enter_context(tc.tile_pool(name="pers", bufs=1))

        # ---- persistent SBUF tensors ----
        wqk_sb = pers.tile([P, CT, 4 * P], f16)  # [c128, ct, (q0,q1,k0,k1)*128]
        wv_sb = pers.tile([P, CT, 2 * P], f16)
        wproj_sb = pers.tile([P, HPC, C], f16)
        cos_sb = pers.tile([P, NTOK], f16)
        sin_sb = pers.tile([P, NTOK], f16)
        mask_sb = pers.tile([P, 1024], f16)
        id_sb = pers.tile([P, P], f16)
        rotm_sb = pers.tile([P, P], f16)
        qT_sb = pers.tile([P, HPC, NTOK], f16)  # [d, h, tok] rope'd
        kT_sb = pers.tile([P, HPC, NTOK], f16)
        v_sb = pers.tile([P, TT, HPC, D + 1], f16)  # [tokmod, tt, h, D|ones]
        oT_sb = pers.tile([P, TT, HPC, P], f16)  # [d, tt, h, tokmod]

        # ---- working pools (all open for the whole kernel: the stack
        # allocator must never reuse a released zone — released-zone deps
        # blow past the 1-wait/instruction HW limit pre-bacc-split) ----
        xt_pool = ctx.enter_context(tc.tile_pool(name="xt", bufs=KNOBS["xt"]))
        rope_pool = ctx.enter_context(tc.tile_pool(name="rope", bufs=KNOBS["rope"]))
        p_pool = ctx.enter_context(tc.tile_pool(name="pt", bufs=KNOBS["pt"]))
        osb_pool = ctx.enter_context(tc.tile_pool(name="osb", bufs=KNOBS["osb"]))
        ysb_pool = ctx.enter_context(tc.tile_pool(name="ysb", bufs=KNOBS["ysb"]))
        # PSUM (8 banks), phase-dedicated to avoid cross-phase slot stalls:
        #   work: 1-bank x3 (B: qk/rot/v chains; D: yps)
        #   cs:   1-bank x3 (C: S-tiles + transposes) — 3 slots decouple the
        #         PE from exp latency by one extra stage
        #   co:   1-bank x2 (C: packed O accumulators, 2 subs each)
        work_pool = ctx.enter_context(tc.tile_pool(name="work", bufs=KNOBS["work"], space="PSUM"))
        cs_pool = ctx.enter_context(tc.tile_pool(name="cs", bufs=KNOBS["cs"], space="PSUM"))
        co_pool = ctx.enter_context(tc.tile_pool(name="co", bufs=KNOBS["co"], space="PSUM"))

        for cth in range(2):
            nc.sync.dma_start(
                wqk_sb[:, cth * 8 : (cth + 1) * 8, :],
                wqk[cth * 1024 : (cth + 1) * 1024, :].rearrange(
                    "(ct p) m -> p ct m", p=P
                ),
            )
        nc.sync.dma_start(wv_sb[:], wv.rearrange("(ct p) m -> p ct m", p=P))
        nc.sync.dma_start(mask_sb[:], maskw[:])
        nc.sync.dma_start(id_sb[:], ident[:])
        nc.sync.dma_start(rotm_sb[:], rotmp[:])
        nc.vector.memset(v_sb[:, :, :, D : D + 1], 1.0)

        # ======== phase B: qkv projection + rope ========
        for ti in range(TOK512):
            t0 = ti * 512
            xt = xt_pool.tile([P, CT, 512], f16, tag="xt")
            for ch in range(4):
                nc.sync.dma_start(
                    xt[:, ch * 4 : (ch + 1) * 4, :],
                    xT[ch * 512 : (ch + 1) * 512, t0 : t0 + 512].rearrange(
                        "(ct p) j -> p ct j", p=P
                    ),
                )
            # stream rope tables alongside
            nc.sync.dma_start(cos_sb[:, t0 : t0 + 512], cos_t[:, t0 : t0 + 512])
            nc.sync.dma_start(sin_sb[:, t0 : t0 + 512], sin_t[:, t0 : t0 + 512])
            # q,k columns: out^T orientation -> [col128, tok512]
            for ci in range(4):
                hh = ci % HPC
                dstT = qT_sb if ci < HPC else kT_sb
                ps = work_pool.tile([P, 512], f32, tag="work", name="psqk")
                for ct in range(CT):
                    nc.tensor.matmul(
                        ps[:],
                        wqk_sb[:, ct, ci * P : (ci + 1) * P],
                        xt[:, ct, :],
                        start=(ct == 0),
                        stop=(ct == CT - 1),
                    )
                st = rope_pool.tile([P, 512], f16, tag="st")
                nc.vector.tensor_copy(st[:], ps[:])
                # half-rotation via PE permutation matmul (no partition-
                # crossing DVE/DMA needed)
                ps2 = work_pool.tile([P, 512], f32, tag="work", name="psrot")
                nc.tensor.matmul(ps2[:], rotm_sb[:], st[:], start=True, stop=True)
                t1 = rope_pool.tile([P, 512], f16, tag="t1")
                t2 = rope_pool.tile([P, 512], f16, tag="t2")
                nc.vector.tensor_mul(t1[:], st[:], cos_sb[:, t0 : t0 + 512])
                nc.vector.tensor_mul(t2[:], ps2[:], sin_sb[:, t0 : t0 + 512])
                nc.vector.tensor_add(dstT[:, hh, t0 : t0 + 512], t1[:], t2[:])
            # v: natural [tok, D*2] orientation (lhsT = xT tile)
            for sub in range(4):
                vps = work_pool.tile([P, 2 * P], f32, tag="work", name="vps")
                for ct in range(CT):
                    nc.tensor.matmul(
                        vps[:],
                        xt[:, ct, sub * P : (sub + 1) * P],
                        wv_sb[:, ct, :],
                        start=(ct == 0),
                        stop=(ct == CT - 1),
                    )
                tt = ti * 4 + sub
                for h in range(HPC):
                    nc.vector.tensor_copy(
                        v_sb[:, tt, h, 0:D], vps[:, h * P : (h + 1) * P]
                    )

        # ======== phase C: causal attention per (b, h) unit ========
        # S^T per k-tile: [k128, q512]. Diagonal k-tiles restricted to the
        # valid causal column range [g, 512). O accumulators packed 2 subs
        # per 1-bank tile (cols 0 and 256).
        for b in range(B):
            for h in range(HPC):
                toff = b * T
                for qi in range(NQ):
                    q0 = toff + qi * 512
                    ndiag0 = qi * 4  # first diagonal kt
                    nkt = ndiag0 + 4
                    # one accumulator per PSUM bank: two interleaved
                    # accumulation groups sharing a bank lose terms on HW
                    o_tiles = [
                        co_pool.tile([P, D + 1], f32, tag="co", name=f"o{_s}")
                        for _s in range(4)
                    ]

                    def pv(pt_ap, kt, sub_lo):
                        for s in range(sub_lo, 4):
                            nc.tensor.matmul(
                                o_tiles[s][:],
                                pt_ap(s),
                                v_sb[:, b * 16 + kt, h, :],
                                start=(kt == 0),
                                stop=(kt == ndiag0 + s),
                            )

                    for kt in range(nkt):
                        k0 = toff + kt * P
                        gi = kt - ndiag0
                        g = max(gi, 0) * P
                        w = 512 - g
                        sd = cs_pool.tile([P, 512], f32, tag="cs", name="sd")
                        nc.tensor.matmul(
                            sd[:, 0:w],
                            kT_sb[:, h, k0 : k0 + P],
                            qT_sb[:, h, q0 + g : q0 + 512],
                            start=True,
                            stop=True,
                        )
                        ptd = p_pool.tile([P, 512], f16, tag="pt", name="ptd")
                        nc.scalar.activation(
                            ptd[:, 0:w], sd[:, 0:w], Exp, scale=float(SCALE)
                        )
                        if gi >= 0:  # diagonal: multiplicative causal mask
                            nc.vector.tensor_mul(
                                ptd[:, 0:w], ptd[:, 0:w], mask_sb[:, 384 : 384 + w]
                            )
                        pv(
                            lambda s, _g=g: ptd[:, s * P - _g : s * P - _g + P],
                            kt,
                            max(gi, 0),
                        )
                    # drain: normalize O rows by 1/rowsum, transpose to O^T
                    for sub in range(4):
                        tt = b * 16 + qi * 4 + sub
                        ot = o_tiles[sub]
                        rtmp = osb_pool.tile([P, 1], f32, tag="rtmp")
                        nc.vector.reciprocal(rtmp[:], ot[:, D : D + 1])
                        o_sb = osb_pool.tile([P, P], f16, tag="osb")
                        if KNOBS["oact"] and sub % 2 == 1:
                            nc.scalar.mul(o_sb[:], ot[:, 0:D], rtmp[:])
                        else:
                            nc.vector.tensor_scalar_mul(o_sb[:], ot[:, 0:D], rtmp[:])
                        tp = cs_pool.tile([P, P], f16, tag="cs", name="tp")
                        nc.tensor.transpose(tp[:], o_sb[:], id_sb[:])
                        if KNOBS["otact"] and sub % 2 == 0:
                            nc.scalar.copy(oT_sb[:, tt, h, :], tp[:])
                        else:
                            nc.vector.tensor_copy(oT_sb[:, tt, h, :], tp[:])

        # deferred wproj load (only needed for phase D)
        nc.sync.dma_start(wproj_sb[:], wproj.rearrange("(h p) m -> p h m", p=P))

        # deferred wproj load (only needed for phase D)
        nc.sync.dma_start(wproj_sb[:], wproj.rearrange("(h p) m -> p h m", p=P))

        # ======== phase D: output projection ========
        for tt in range(TT):
            for cc in range(4):
                yps = work_pool.tile([P, 512], f32, tag="work", name="yps")
                for h in range(HPC):
                    nc.tensor.matmul(
                        yps[:],
                        oT_sb[:, tt, h, :],
                        wproj_sb[:, h, cc * 512 : (cc + 1) * 512],
                        start=(h == 0),
                        stop=(h == HPC - 1),
                    )
                ysb = ysb_pool.tile([P, 512], f16, tag="ysb")
                if cc % KNOBS["dact"] == 0:
                    nc.scalar.copy(ysb[:], yps[:])
                else:
                    nc.vector.tensor_copy(ysb[:], yps[:])
                nc.sync.dma_start(
                    y[tt * P : (tt + 1) * P, cc * 512 : (cc + 1) * 512], ysb[:]
                )

    # bacc lowering: splits multi-sem waits into EventSemaphore insts
    # (TRN2 allows at most 1 wait per regular instruction), reg alloc, DCE.
    nc.compile()
    return nc


def _host_inputs(x, Wqkv, Wproj):
    """Build per-core device input maps (host-side sharding)."""
    xTf = np.ascontiguousarray(x.reshape(NTOK, C).T).astype(np.float16)

    invf = 1.0 / (10000.0 ** (np.arange(0, D, 2, dtype=np.float32) / D))
    freqs = np.arange(T, dtype=np.float32)[:, None] * invf[None, :]  # [T, 64]
    cos = np.cos(freqs).astype(np.float32).T  # [64, T]
    sin = np.sin(freqs).astype(np.float32).T
    cos_t = np.tile(np.concatenate([cos, cos], axis=0), (1, B)).astype(np.float16)
    sin_t = np.tile(np.concatenate([-sin, sin], axis=0), (1, B)).astype(np.float16)

    ii = np.arange(P)[:, None]
    mm = np.arange(1024)[None, :]
    maskw = (mm >= ii + 384).astype(np.float16)
    ident = np.eye(P, dtype=np.float16)
    rotm = np.zeros((P, P), dtype=np.float16)
    rotm[(np.arange(P) + 64) % P, np.arange(P)] = 1.0

    in_maps = []
    for c in range(NCORES):
        h0 = c * HPC * D  # col offset of this core's heads
        wqk_c = np.concatenate(
            [Wqkv[:, h0 : h0 + HPC * D], Wqkv[:, C + h0 : C + h0 + HPC * D]], axis=1
        ).astype(np.float16)
        wv_c = Wqkv[:, 2 * C + h0 : 2 * C + h0 + HPC * D].astype(np.float16)
        wproj_c = np.ascontiguousarray(Wproj[h0 : h0 + HPC * D, :]).astype(np.float16)
        in_maps.append(
            {
                "xT": xTf,
                "wqk": np.ascontiguousarray(wqk_c),
                "wv": np.ascontiguousarray(wv_c),
                "wproj": wproj_c,
                "cos_t": cos_t,
                "sin_t": sin_t,
                "maskw": maskw,
                "ident": ident,
                "rotm": rotm,
            }
        )
    return in_maps


def kernel(x, Wqkv, Wproj, _trace=False):
    global _compiled
    x = np.asarray(x, dtype=np.float32)
    Wqkv = np.asarray(Wqkv, dtype=np.float32)
    Wproj = np.asarray(Wproj, dtype=np.float32)

    from concourse.bass_utils import run_bass_kernel_spmd

    if _compiled is None:
        _compiled = _build_bass()
    nc = _compiled

    in_maps = _host_inputs(x, Wqkv, Wproj)
    res = run_bass_kernel_spmd(nc, in_maps, list(range(NCORES)), trace=_trace)
    out = np.zeros((NTOK, C), dtype=np.float32)
    for r in res.results:
        out += r["y"].astype(np.float32)
    kernel._last_result = res
    return out.reshape(B, T, C)



# revision 39
# speedup vs baseline: 1.1433x; 1.1433x over previous
"""Causal self-attention (B=2, T=2048, C=2048, H=16, rope) on 8 trn2 cores.

Sharding: tensor-parallel over heads. Each core owns 2 of 16 heads:
  - Wqkv columns for its heads (q,k,v), Wproj rows for its heads.
  - Computes qkv projection, rope, causal attention, and its partial
    output projection y_c (full [4096, 2048]).
  - Host sums the 8 partials (the all-reduce / unshard for row-parallel TP).

Large GEMMs (qkv projection, output projection) run as fp8(e4m3)
DoubleRow matmuls (0.5 cycles/row, 256-deep contraction: 4x the fp16 PE
rate) with residual error-compensation:
    x @ W ~= x8@W8 + xr@W8 + x8@Wr
where x8 = q8(x*s), xr = q8(x*s - x8) (all planes share one power-of-2
scale so the three terms accumulate in a single PSUM group; the combined
2^-11 is folded into downstream constants). Measured end-to-end rel err
~3e-3. The DoubleRow pair dim packs either (even,odd) contraction blocks
(main term) or (hi,lo)x(lo,hi) planes (correction term, one instruction
per 128-block).

Attention itself stays fp16 (K=128 contraction gains nothing from
DoubleRow; softmax accuracy matters). Softmax sums come free as a
ones-column appended to V; normalization is applied to O before
projection. O is quantized on-chip to fp8 hi+lo planes for the fp8
output projection. Phases are interleaved per batch (qkv b0, attn b0
with the projection of each q-block right after its second head,
qkv b1, ...) so y DMAs and drains spread across the whole kernel.
"""

import sys

for _p in ("/opt/trn_rl_repo",):
    if _p not in sys.path:
        sys.path.append(_p)

import numpy as np

# ---- problem constants (hardcoded per the task contract) ----
B, T, C, H = 2, 2048, 2048, 16
D = C // H  # 128
NCORES = 8
HPC = H // NCORES  # heads per core = 2
NTOK = B * T  # 4096
P = 128
CT = C // P  # 16 contraction 128-blocks
TPB = T // 512  # 512-token tiles per batch = 4
NQ = T // 512  # q-tiles per unit = 4
TT = NTOK // P  # 32 token 128-tiles
SCALE = 1.0 / np.sqrt(D)

# fp8 power-of-2 scales (shared by hi and lo planes of each tensor)
SX = 32.0  # x
SWQ = 64.0  # Wqkv
SO = 32.0  # attention output O
SWP = 64.0  # Wproj
INV_QKV = 1.0 / (SX * SWQ)  # 1/2048, folded into rope tables / ones col
INV_PROJ = 1.0 / (SO * SWP)  # 1/2048, folded into the y copy
# ones column value: makes o16 = O_psum * 1/(ONES*sum p) come out as SO*O
ONES_VAL = SX * SWQ / SO  # 64

_compiled = None

# tuning knobs (overridable before _build_bass for experiments)
KNOBS = {
    "xt": 2,  # x-tile double buffering
    "rope": 4,
    "pt": 6,
    "osb": 8,
    "ysb": 8,
    "work": 4,  # PSUM banks: qkv ps / rot ps2 / vps / yps chains
    "cs": 2,  # PSUM banks: S tiles + O transposes
    "co": 2,  # PSUM banks: O accumulator pair-banks (2 subs per bank)
    "o16_eng": 0,  # O normalize engine: 0=DVE 1=Act  (PSUM: no Pool)
    "st_act": 1,  # rope staging copy: 0=DVE 1=Act 2=alternate
    "v_act": 0,  # v psum->sbuf copies on scalar engine
    "lag": 2,  # attention S/exp pipeline depth ahead of PV
    "ypat": "DA",  # y-copy engine rotation (D=DVE A=Act; PSUM: no Pool)
}


def _build_bass():
    import concourse.bacc as bacc
    import concourse.mybir as mybir
    import concourse.tile as tile
    from contextlib import ExitStack

    f8 = mybir.dt.float8e4
    f16 = mybir.dt.float16
    f32 = mybir.dt.float32
    Exp = mybir.ActivationFunctionType.Exp
    DR = mybir.MatmulPerfMode.DoubleRow

    nc = bacc.Bacc()

    # fp8 plane conventions:
    #   xall:  plane0 = x8 (hi), plane1 = xr (lo)
    #   wqkI:  plane0 = Wr (lo), plane1 = W8 (hi)   [same for wvI]
    #   wpI:   plane0 = Wp8 (hi), plane1 = Wpr (lo)
    #   oT_sb: plane0 = olo, plane1 = ohi
    # main terms pair (even,odd) hi-blocks; corrections pair planes so one
    # DoubleRow instruction computes lo.T@hi + hi.T@lo for a 128-block.
    xall = nc.declare_dram_parameter("xall", [P, CT, 2, NTOK], f8, isOutput=False)
    wqkI = nc.declare_dram_parameter("wqkI", [P, 4, CT, 2, P], f8, isOutput=False)
    wvI = nc.declare_dram_parameter("wvI", [P, CT, 2, 2 * P], f8, isOutput=False)
    wpI = nc.declare_dram_parameter("wpI", [P, HPC, 2, C], f8, isOutput=False)
    cos_t = nc.declare_dram_parameter("cos_t", [P, NTOK], f16, isOutput=False)
    sin_t = nc.declare_dram_parameter("sin_t", [P, NTOK], f16, isOutput=False)
    # consts packs [maskw(1024) | ident(128) | rotm(128)] to make the
    # startup-critical loads a single DMA
    consts = nc.declare_dram_parameter("consts", [P, 1280], f16, isOutput=False)
    y = nc.declare_dram_parameter("y", [NTOK, C], f16, isOutput=True)

    with tile.TileContext(nc) as tc, ExitStack() as ctx:
        pers = ctx.enter_context(tc.tile_pool(name="pers", bufs=1))

        # ---- persistent SBUF tensors ----
        wqk_sb = pers.tile([P, 4, CT, 2, P], f8)
        wv_sb = pers.tile([P, CT, 2, 2 * P], f8)
        wp_sb = pers.tile([P, HPC, 2, C], f8)
        cos_sb = pers.tile([P, NTOK], f16)
        sin_sb = pers.tile([P, NTOK], f16)
        const_sb = pers.tile([P, 1280], f16)
        mask_sb = const_sb[:, 0:1024]
        id_sb = const_sb[:, 1024:1152]
        rotm_sb = const_sb[:, 1152:1280]
        qT_sb = pers.tile([P, HPC, NTOK], f16)  # [d, h, tok] rope'd
        kT_sb = pers.tile([P, HPC, NTOK], f16)
        v_sb = pers.tile([P, TT, HPC, D + 1], f16)  # [tokmod, tt, h, 2048*V|64]
        oT_sb = pers.tile([P, TT, HPC, 2, P], f8)  # [d, tt, h, (olo,ohi), tokmod]

        # ---- working pools (all open for the whole kernel: the stack
        # allocator must never reuse a released zone — released-zone deps
        # blow past the 1-wait/instruction HW limit pre-bacc-split) ----
        xt_pool = ctx.enter_context(tc.tile_pool(name="xt", bufs=KNOBS["xt"]))
        rope_pool = ctx.enter_context(tc.tile_pool(name="rope", bufs=KNOBS["rope"]))
        p_pool = ctx.enter_context(tc.tile_pool(name="pt", bufs=KNOBS["pt"]))
        osb_pool = ctx.enter_context(tc.tile_pool(name="osb", bufs=KNOBS["osb"]))
        ysb_pool = ctx.enter_context(tc.tile_pool(name="ysb", bufs=KNOBS["ysb"]))
        # PSUM (8 banks): work 2 + cs 2 + co 4. Each accumulation group owns
        # a full bank (two interleaved groups sharing a bank lose terms on HW).
        work_pool = ctx.enter_context(
            tc.tile_pool(name="work", bufs=KNOBS["work"], space="PSUM")
        )
        cs_pool = ctx.enter_context(
            tc.tile_pool(name="cs", bufs=KNOBS["cs"], space="PSUM")
        )
        co_pool = ctx.enter_context(
            tc.tile_pool(name="co", bufs=KNOBS["co"], space="PSUM")
        )

        # constants + the first weight column first: the leading xt DMA
        # follows immediately so the PE can start ~7us in.
        nc.sync.dma_start(const_sb[:], consts[:])
        nc.sync.dma_start(wqk_sb[:, 0], wqkI[:, 0])
        nc.vector.memset(v_sb[:, :, :, D : D + 1], ONES_VAL)

        def load_xtile(b, ti):
            t0 = b * T + ti * 512
            xt = xt_pool.tile([P, CT, 2, 512], f8, tag="xt", name=f"xt{b}{ti}")
            nc.sync.dma_start(xt[:], xall[:, :, :, t0 : t0 + 512])
            nc.sync.dma_start(cos_sb[:, t0 : t0 + 512], cos_t[:, t0 : t0 + 512])
            nc.sync.dma_start(sin_sb[:, t0 : t0 + 512], sin_t[:, t0 : t0 + 512])
            return xt

        def qk_chain(b, ti, xt, ci):
            """One q/k column chain of the qkv projection + its rope."""
            t0 = b * T + ti * 512
            hh = ci % HPC
            dstT = qT_sb if ci < HPC else kT_sb
            ps = work_pool.tile([P, 512], f32, tag="work", name="psqk")
            for tp in range(CT // 2):
                nc.tensor.matmul(
                    ps[:],
                    wqk_sb[:, ci, 2 * tp : 2 * tp + 2, 1, :],
                    xt[:, 2 * tp : 2 * tp + 2, 0, :],
                    start=(tp == 0),
                    stop=False,
                    perf_mode=DR,
                )
            for ct in range(CT):
                nc.tensor.matmul(
                    ps[:],
                    wqk_sb[:, ci, ct, 0:2, :],
                    xt[:, ct, 0:2, :],
                    start=False,
                    stop=(ct == CT - 1),
                    perf_mode=DR,
                )
            st = rope_pool.tile([P, 512], f16, tag="st")
            st_act = KNOBS["st_act"]
            if st_act == 2:
                st_act = ci % 2
            if st_act:
                nc.scalar.copy(st[:], ps[:])
            else:
                nc.vector.tensor_copy(st[:], ps[:])
            # half-rotation via PE permutation matmul
            ps2 = work_pool.tile([P, 512], f32, tag="work", name="psrot")
            nc.tensor.matmul(ps2[:], rotm_sb[:], st[:], start=True, stop=True)
            t1 = rope_pool.tile([P, 512], f16, tag="t1")
            t2 = rope_pool.tile([P, 512], f16, tag="t2")
            # t1 and the final add are all-SBUF -> Pool engine (GPSIMD
            # cannot touch PSUM, so t2 and the staging copy cannot move)
            nc.gpsimd.tensor_mul(t1[:], st[:], cos_sb[:, t0 : t0 + 512])
            nc.vector.tensor_mul(t2[:], ps2[:], sin_sb[:, t0 : t0 + 512])
            nc.gpsimd.tensor_add(dstT[:, hh, t0 : t0 + 512], t1[:], t2[:])

        def v_chain(b, ti, xt, sub):
            """One 128-token v chain of the qkv projection."""
            sl = slice(sub * P, (sub + 1) * P)
            vps = work_pool.tile([P, 2 * P], f32, tag="work", name="vps")
            for tp in range(CT // 2):
                nc.tensor.matmul(
                    vps[:],
                    xt[:, 2 * tp : 2 * tp + 2, 0, sl],
                    wv_sb[:, 2 * tp : 2 * tp + 2, 1, :],
                    start=(tp == 0),
                    stop=False,
                    perf_mode=DR,
                )
            for ct in range(CT):
                nc.tensor.matmul(
                    vps[:],
                    xt[:, ct, 0:2, sl],
                    wv_sb[:, ct, 0:2, :],
                    start=False,
                    stop=(ct == CT - 1),
                    perf_mode=DR,
                )
            tt = (b * TPB + ti) * 4 + sub
            for h in range(HPC):
                if KNOBS["v_act"]:
                    nc.scalar.copy(v_sb[:, tt, h, 0:D], vps[:, h * P : (h + 1) * P])
                else:
                    nc.vector.tensor_copy(
                        v_sb[:, tt, h, 0:D], vps[:, h * P : (h + 1) * P]
                    )

        def b_chunks(b, ti, xt):
            """The 8 PE-filler chunks of one 512-token qkv tile."""
            out = []
            for ci in range(4):
                out.append(lambda ci=ci: qk_chain(b, ti, xt, ci))
            for sub in range(4):
                out.append(lambda sub=sub: v_chain(b, ti, xt, sub))
            return out

        def attn_unit(b, h, qi, fillers):
            """One (batch, head, q-block) causal attention unit.

            Software-pipelined: S(kt) + exp(kt) run 2 kt ahead of PV(kt), so
            exp latency hides behind queued PE work; the 2-slot sd pool
            paces S production to the Act engine. Next-tile qkv chains
            (fillers) are spliced between steps to keep the PE fed while
            Act drains its exp backlog. Each O accumulator is drained as
            soon as its accumulation stops."""
            toff = b * T
            q0 = toff + qi * 512
            ndiag0 = qi * 4  # first diagonal kt
            nkt = ndiag0 + 4
            lag = KNOBS["lag"]
            # two subs share one accumulation bank: a single start=True at
            # kt=0 zeroes the bank, then both subs accumulate into disjoint
            # column ranges (one group per bank; per-sub stop flags with the
            # group check disabled).
            o_pairs = [
                co_pool.tile([P, 512], f32, tag="co", name=f"op{_j}")
                for _j in range(2)
            ]

            def ot(s):
                return o_pairs[s // 2][:, (s % 2) * (D + 1) : (s % 2 + 1) * (D + 1)]

            ptd_tiles = [None] * nkt
            stride = max(2, (nkt + lag) // 6)

            def s_exp(kt):
                k0 = toff + kt * P
                gi = kt - ndiag0
                g = max(gi, 0) * P
                w = 512 - g
                sd = cs_pool.tile([P, 512], f32, tag="cs", name="sd")
                nc.tensor.matmul(
                    sd[:, 0:w],
                    kT_sb[:, h, k0 : k0 + P],
                    qT_sb[:, h, q0 + g : q0 + 512],
                    start=True,
                    stop=True,
                )
                ptd = p_pool.tile([P, 512], f16, tag="pt", name="ptd")
                ptd_tiles[kt] = ptd
                nc.scalar.activation(ptd[:, 0:w], sd[:, 0:w], Exp, scale=float(SCALE))
                if gi >= 0:
                    # diagonal: multiplicative causal mask. Only the first
                    # 128 columns (j < i possible only there) need it, so
                    # the later subs' PV matmuls don't depend on the mask.
                    # All-SBUF -> Pool engine.
                    nc.gpsimd.tensor_mul(
                        ptd[:, 0:P], ptd[:, 0:P], mask_sb[:, 384 : 384 + P]
                    )

            def pv(kt):
                gi = kt - ndiag0
                g = max(gi, 0) * P
                ptd = ptd_tiles[kt]
                for s in range(max(gi, 0), 4):
                    nc.tensor.matmul(
                        ot(s),
                        ptd[:, s * P - g : s * P - g + P],
                        v_sb[:, b * 16 + kt, h, :],
                        start=(kt == 0 and s % 2 == 0),
                        stop=(kt == ndiag0 + s),
                        skip_group_check=True,
                    )

            def drain(sub):
                tt = b * 16 + qi * 4 + sub
                oa = ot(sub)
                rtmp = osb_pool.tile([P, 1], f32, tag="rtmp")
                nc.vector.reciprocal(rtmp[:], oa[:, D : D + 1])
                o16 = osb_pool.tile([P, P], f16, tag="osb")
                if KNOBS["o16_eng"] == 1:
                    nc.scalar.mul(o16[:], oa[:, 0:D], rtmp[:])
                else:
                    nc.vector.tensor_scalar_mul(o16[:], oa[:, 0:D], rtmp[:])
                oT16 = cs_pool.tile([P, P], f16, tag="cs", name="tp")
                nc.tensor.transpose(oT16[:], o16[:], id_sb[:])
                nc.scalar.copy(oT_sb[:, tt, h, 1, :], oT16[:])
                nc.vector.tensor_sub(
                    oT_sb[:, tt, h, 0, :], oT16[:], oT_sb[:, tt, h, 1, :]
                )

            for step in range(nkt + lag):
                if step < nkt:
                    s_exp(step)
                kt = step - lag
                if kt >= 0:
                    pv(kt)
                    if kt >= ndiag0:
                        drain(kt - ndiag0)
                if fillers and qi >= 1 and step % stride == stride - 1:
                    fillers.pop(0)()

        ycnt = [0]

        def proj_mini(b, qi, sub):
            """Output projection for one token 128-tile. The PSUM->SBUF y
            staging round-robins over Pool/DVE/Act."""
            tt = b * 16 + qi * 4 + sub
            for cc in range(4):
                ccs = slice(cc * 512, (cc + 1) * 512)
                yps = work_pool.tile([P, 512], f32, tag="work", name="yps")
                nc.tensor.matmul(
                    yps[:],
                    oT_sb[:, tt, 0:2, 1, :],
                    wp_sb[:, 0:2, 0, ccs],
                    start=True,
                    stop=False,
                    perf_mode=DR,
                )
                for h in range(HPC):
                    nc.tensor.matmul(
                        yps[:],
                        oT_sb[:, tt, h, 0:2, :],
                        wp_sb[:, h, 0:2, ccs],
                        start=False,
                        stop=(h == HPC - 1),
                        perf_mode=DR,
                    )
                ysb = ysb_pool.tile([P, 512], f16, tag="ysb")
                pat = KNOBS["ypat"]
                eng = pat[ycnt[0] % len(pat)]
                ycnt[0] += 1
                if eng == "D":
                    nc.vector.tensor_scalar_mul(ysb[:], yps[:], INV_PROJ)
                else:
                    nc.scalar.mul(ysb[:], yps[:], INV_PROJ)
                nc.sync.dma_start(y[tt * P : (tt + 1) * P, ccs], ysb[:])

        def proj_chunks(b, qi):
            return [lambda s=s: proj_mini(b, qi, s) for s in range(4)]

        # ======== interleaved schedule ========
        # Causality lets attention on q-block qi start right after qkv
        # tile ti=qi. Super-iteration k runs attention on tile k while the
        # qkv chains of tile k+1 and the (delayed) projection of tile k-1
        # fill PE gaps; the x for tile k+2 prefetches ahead of the y DMAs.
        tiles = [(b, ti) for b in range(B) for ti in range(TPB)]
        xts = {0: load_xtile(*tiles[0])}
        for ci in range(1, 4):
            nc.sync.dma_start(wqk_sb[:, ci], wqkI[:, ci])
        nc.sync.dma_start(wv_sb[:], wvI[:])
        for ch in b_chunks(*tiles[0], xts[0]):
            ch()
        xts[1] = load_xtile(*tiles[1])
        nc.sync.dma_start(wp_sb[:], wpI[:])
        for k, (b, qi) in enumerate(tiles):
            fillers = []
            if k + 1 < len(tiles):
                fillers += b_chunks(*tiles[k + 1], xts[k + 1])
            if k >= 1:
                pb, pqi = tiles[k - 1]
                fillers += proj_chunks(pb, pqi)
            for h in range(HPC):
                attn_unit(b, h, qi, fillers)
            if k + 2 < len(tiles):
                xts[k + 2] = load_xtile(*tiles[k + 2])
            while fillers:
                fillers.pop(0)()
        for ch in proj_chunks(*tiles[-1]):
            ch()

    # bacc lowering: splits multi-sem waits into EventSemaphore insts
    # (TRN2 allows at most 1 wait per regular instruction), reg alloc, DCE.
    nc.compile()
    return nc


def _host_inputs(x, Wqkv, Wproj):
    """Build per-core device input maps (host-side sharding + fp8 split)."""
    import ml_dtypes

    E4 = ml_dtypes.float8_e4m3fn

    def split8(a, s):
        hi = (a * s).astype(E4)
        lo = ((a * s) - hi.astype(np.float32)).astype(E4)
        return hi, lo

    xT = np.ascontiguousarray(x.reshape(NTOK, C).T)  # [C, NTOK] f32
    x_hi, x_lo = split8(xT, SX)
    xall = np.empty((P, CT, 2, NTOK), dtype=E4)
    xall[:, :, 0, :] = x_hi.reshape(CT, P, NTOK).transpose(1, 0, 2)
    xall[:, :, 1, :] = x_lo.reshape(CT, P, NTOK).transpose(1, 0, 2)

    invf = 1.0 / (10000.0 ** (np.arange(0, D, 2, dtype=np.float32) / D))
    freqs = np.arange(T, dtype=np.float32)[:, None] * invf[None, :]  # [T, 64]
    cos = np.cos(freqs).astype(np.float32).T * INV_QKV  # [64, T]
    sin = np.sin(freqs).astype(np.float32).T * INV_QKV
    cos_t = np.tile(np.concatenate([cos, cos], axis=0), (1, B)).astype(np.float16)
    sin_t = np.tile(np.concatenate([-sin, sin], axis=0), (1, B)).astype(np.float16)

    ii = np.arange(P)[:, None]
    mm = np.arange(1024)[None, :]
    maskw = (mm >= ii + 384).astype(np.float16)
    ident = np.eye(P, dtype=np.float16)
    rotm = np.zeros((P, P), dtype=np.float16)
    rotm[(np.arange(P) + 64) % P, np.arange(P)] = 1.0
    consts = np.concatenate([maskw, ident, rotm], axis=1)  # [P, 1280]

    in_maps = []
    for c in range(NCORES):
        h0 = c * HPC * D  # col offset of this core's heads
        wqk_c = np.concatenate(
            [Wqkv[:, h0 : h0 + HPC * D], Wqkv[:, C + h0 : C + h0 + HPC * D]], axis=1
        )  # [C, 512] (q0,q1,k0,k1)
        qk_hi, qk_lo = split8(wqk_c, SWQ)
        wqkI = np.empty((P, 4, CT, 2, P), dtype=E4)
        wqkI[:, :, :, 0, :] = qk_lo.reshape(CT, P, 4, P).transpose(1, 2, 0, 3)
        wqkI[:, :, :, 1, :] = qk_hi.reshape(CT, P, 4, P).transpose(1, 2, 0, 3)

        wv_c = Wqkv[:, 2 * C + h0 : 2 * C + h0 + HPC * D]  # [C, 256]
        v_hi, v_lo = split8(wv_c, SWQ)
        wvI = np.empty((P, CT, 2, 2 * P), dtype=E4)
        wvI[:, :, 0, :] = v_lo.reshape(CT, P, 2 * P).transpose(1, 0, 2)
        wvI[:, :, 1, :] = v_hi.reshape(CT, P, 2 * P).transpose(1, 0, 2)

        wp_c = Wproj[h0 : h0 + HPC * D, :]  # [256, C]
        p_hi, p_lo = split8(wp_c, SWP)
        wpI = np.empty((P, HPC, 2, C), dtype=E4)
        wpI[:, :, 0, :] = p_hi.reshape(HPC, P, C).transpose(1, 0, 2)
        wpI[:, :, 1, :] = p_lo.reshape(HPC, P, C).transpose(1, 0, 2)

        in_maps.append(
            {
                "xall": xall,
                "wqkI": np.ascontiguousarray(wqkI),
                "wvI": np.ascontiguousarray(wvI),
                "wpI": np.ascontiguousarray(wpI),
                "cos_t": cos_t,
                "sin_t": sin_t,
                "consts": consts,
            }
        )
    return in_maps


def kernel(x, Wqkv, Wproj, _trace=False):
    global _compiled
    x = np.asarray(x, dtype=np.float32)
    Wqkv = np.asarray(Wqkv, dtype=np.float32)
    Wproj = np.asarray(Wproj, dtype=np.float32)

    from concourse.bass_utils import run_bass_kernel_spmd

    if _compiled is None:
        _compiled = _build_bass()
    nc = _compiled

    in_maps = _host_inputs(x, Wqkv, Wproj)
    res = run_bass_kernel_spmd(nc, in_maps, list(range(NCORES)), trace=_trace)
    out = np.zeros((NTOK, C), dtype=np.float32)
    for r in res.results:
        out += r["y"].astype(np.float32)
    kernel._last_result = res
    return out.reshape(B, T, C)
